# revision 1
# baseline (speedup 1.0000x reference)
"""Trainium2 Bass kernel for a GQA attention block (B=2, S=2048, H=2048,
16 q-heads / 8 kv-heads, head_dim=128, fp32), tensor-parallel over heads
across 8 NeuronCores.

Per-core shard (core c): q-heads {2c, 2c+1}, kv-head c; wq/wk/wv column
shards, wo row shard. x is replicated (pre-transposed on host so the
contraction dim lands on SBUF partitions). Each core emits a partial
[4096, 2048] o-proj product; the host gather for the row-parallel o-proj
is a sum over the 8 partials.

Device dataflow (per core):
  A) QKV^T projections ([d, tok] layout) via float32r matmuls; one ACT
     copy evicts each PSUM head slab to SBUF; RMSNorm sum-of-squares via
     GPSIMD partition-allreduce (the q/k norm weights are folded into the
     RoPE tables on the host); RoPE as partition-half shuffle; the rstd
     scale is applied after RoPE (commutes -- rstd is column-uniform).
     V is transposed back to natural [tok, d] via PE transposes.
  B) Causal attention, two sub-phases per (batch, q-tile, head):
     (1) S^T tiles [128 k, 512 q] = K^T_tile.T @ Q^T + exp on ACT (no max
         subtraction -- RMSNorm bounds |scores| <= sqrt(128)) + causal
         affine_select on the diagonal band;
     (2) softmax denominator (ones-vector matmuls) and PV (V_nat as
         stationary) accumulated over k-tiles.
     Then the row-parallel o-proj partial, streamed out per 512-row tile.
"""

import math
import os
import sys

import numpy as np

for _p in ("/opt/trn_rl_repo", "/root/.axon_site/_ro/trn_rl_repo"):
    if os.path.isdir(_p) and _p not in sys.path:
        sys.path.insert(0, _p)
        break

import concourse.bacc as bacc
import concourse.tile as tile
from concourse import mybir
from concourse.bass_isa import ReduceOp
from concourse.bass_utils import run_bass_kernel_spmd
from concourse.masks import make_identity

# Problem constants (hardcoded per contract)
B, S, HID = 2, 2048, 2048
NH, NKV, D = 16, 8, 128
NCORES = 8
HQ = NH // NCORES  # q heads per core = 2
T = B * S          # 4096 tokens
EPS = 1e-5
F32 = mybir.dt.float32
F32R = mybir.dt.float32r
BF16 = mybir.dt.bfloat16
# matmul input dtype: "f32r" (near-fp32, default) or "bf16" (halves phase-A
# DMA; ~1e-3-class output error)
KDT = os.environ.get("BASS_KDT", "f32r")
MDT = BF16 if KDT == "bf16" else F32R
NP_MDT = None  # set lazily in prep_inputs (ml_dtypes import)
# transpose path (identity matmul) dtype: f32r can't be memset/ldweights'd,
# so use plain f32 there in f32r mode
TDT = BF16 if KDT == "bf16" else F32
SCALE = 1.0 / math.sqrt(D)

KT = HID // 128      # 16 contraction tiles
TT = T // 512        # 8 token tiles of 512
QT_PER_B = S // 512  # 4 q-tiles per batch


def build_nc():
    nc = bacc.Bacc("TRN2", target_bir_lowering=False, debug=False)
    xt = nc.dram_tensor("xt", [HID, T], MDT, kind="ExternalInput").ap()
    wqkv = nc.dram_tensor("wqkv", [HID, 4 * D], MDT, kind="ExternalInput").ap()
    woc = nc.dram_tensor("woc", [HQ * D, HID], MDT, kind="ExternalInput").ap()
    pmat = nc.dram_tensor("pmat", [D, D], MDT, kind="ExternalInput").ap()
    onec = nc.dram_tensor("onec", [D, 1], MDT, kind="ExternalInput").ap()
    ctq = nc.dram_tensor("ctq", [D, S], MDT, kind="ExternalInput").ap()
    stq = nc.dram_tensor("stq", [D, S], MDT, kind="ExternalInput").ap()
    ctk = nc.dram_tensor("ctk", [D, S], MDT, kind="ExternalInput").ap()
    stk = nc.dram_tensor("stk", [D, S], MDT, kind="ExternalInput").ap()
    out = nc.dram_tensor("out", [T, HID], F32, kind="ExternalOutput").ap()

    with tile.TileContext(nc) as tc:
        from contextlib import ExitStack

        with ExitStack() as root:
            const = root.enter_context(tc.tile_pool(name="const", bufs=1))
            ident = const.tile([128, 128], TDT, name="ident")
            make_identity(nc, ident)
            ones_col = const.tile([128, 1], MDT, name="ones_col")
            nc.scalar.dma_start(out=ones_col, in_=onec)
            pmat_sb = const.tile([D, D], MDT, name="pmat_sb")
            nc.scalar.dma_start(out=pmat_sb, in_=pmat)
            eps_col = const.tile([128, 1], F32, name="eps_col")
            nc.vector.memset(eps_col, EPS)

            res = root.enter_context(tc.tile_pool(name="res", bufs=1))
            wo_sb = res.tile([128, HQ, HID], MDT, name="wo_sb")
            qt_sb = res.tile([128, HQ, T], MDT, name="qt_sb")   # [d, h, tok]
            kt_sb = res.tile([128, T], MDT, name="kt_sb")       # [d, tok]
            v_sb = res.tile([128, T // 128, D], MDT, name="v_sb")  # [tok%128, tile, d]

            # ---------------- Phase A: QKV^T, norm, rope, V transpose ---------
            with ExitStack() as pa:
                wqp = pa.enter_context(tc.tile_pool(name="wqp", bufs=1))
                xp = pa.enter_context(tc.tile_pool(name="xp", bufs=17))
                tabp = pa.enter_context(tc.tile_pool(name="tabp", bufs=2))
                wp = pa.enter_context(tc.tile_pool(name="wp", bufs=2))
                psA = pa.enter_context(tc.tile_pool(name="psA", bufs=2, space="PSUM"))
                psT = pa.enter_context(tc.tile_pool(name="psT", bufs=2, space="PSUM"))
                psR = pa.enter_context(tc.tile_pool(name="psR", bufs=2, space="PSUM"))

                wqkv_sb = wqp.tile([128, KT, 4 * D], MDT, name="wqkv_sb")

                # visit token tiles as (b0, b1) pairs sharing a sequence
                # position so each RoPE table slice is fetched once
                tabs = {}
                for ti, t in enumerate((0, 4, 1, 5, 2, 6, 3, 7)):
                    xks = []
                    for k in range(KT):
                        if ti == 0:  # interleave weight loads with first x tiles
                            nc.sync.dma_start(
                                out=wqkv_sb[:, k, :], in_=wqkv[k * 128:(k + 1) * 128, :]
                            )
                        xk = xp.tile([128, 512], MDT, name="xk", tag="xk")
                        nc.sync.dma_start(
                            out=xk, in_=xt[k * 128:(k + 1) * 128, t * 512:(t + 1) * 512]
                        )
                        xks.append(xk)
                    if ti == 5:  # wo is not needed until phase B
                        nc.sync.dma_start(
                            out=wo_sb, in_=woc.rearrange("(h p) n -> p h n", p=128)
                        )
                    # two 2-bank PSUM slabs: (q0,q1) and (k,v)
                    slabs = []
                    for g in range(2):
                        ps = psA.tile([128, 2, 512], F32, name="ps_qkv", tag="ps_qkv")
                        for k in range(KT):
                            for mm in range(2):
                                m = g * 2 + mm
                                nc.tensor.matmul(
                                    ps[:, mm, :],
                                    lhsT=(wqkv_sb[:, k, m * 128:(m + 1) * 128]),
                                    rhs=(xks[k]),
                                    start=(k == 0),
                                    stop=(k == KT - 1),
                                )
                        slabs.append(ps)

                    s0 = (t % QT_PER_B) * 512  # position-in-sequence of this tile
                    if ti % 2 == 0:  # second tile of each pair reuses the slices
                        tabs = {}
                        for nm, ap in (("cq", ctq), ("sq", stq), ("ck", ctk), ("sk", stk)):
                            tl = tabp.tile([128, 512], MDT, name="tab_" + nm, tag="tab_" + nm)
                            nc.sync.dma_start(out=tl, in_=ap[:, s0:s0 + 512])
                            tabs[nm] = tl
                    for m, cosT, sinT in (
                        (0, tabs["cq"], tabs["sq"]),
                        (1, tabs["cq"], tabs["sq"]),
                        (2, tabs["ck"], tabs["sk"]),
                    ):
                        src = slabs[m // 2][:, m % 2, :]
                        qk = wp.tile([128, 512], MDT, name="qk", tag="qk")
                        nc.scalar.copy(qk, src)  # sole PSUM reader (ACT)
                        sq = wp.tile([128, 512], F32, name="sq", tag="sq")
                        nc.vector.tensor_mul(sq, qk, qk)
                        nc.gpsimd.partition_all_reduce(sq, sq, 128, ReduceOp.add)
                        rrow = wp.tile([1, 512], F32, name="rrow", tag="rrow")
                        nc.scalar.activation(
                            rrow, sq[0:1, :], mybir.ActivationFunctionType.Sqrt,
                            bias=eps_col[0:1, :], scale=1.0 / D,
                        )
                        nc.vector.reciprocal(rrow, rrow)
                        rstd = wp.tile([128, 512], F32, name="rstd", tag="rstd")
                        nc.gpsimd.partition_broadcast(rstd, rrow)
                        shf = psR.tile([128, 512], F32, name="shf", tag="shf")
                        nc.tensor.matmul(shf, lhsT=pmat_sb, rhs=qk, start=True, stop=True)
                        t0 = wp.tile([128, 512], F32, name="t0", tag="t0")
                        nc.vector.tensor_mul(t0, qk, cosT)
                        t1 = wp.tile([128, 512], F32, name="t1", tag="t1")
                        nc.vector.tensor_mul(t1, shf, sinT)
                        tr = wp.tile([128, 512], F32, name="tr", tag="tr")
                        nc.vector.tensor_add(tr, t0, t1)
                        if m < 2:
                            dst = qt_sb[:, m, t * 512:(t + 1) * 512]
                        else:
                            dst = kt_sb[:, t * 512:(t + 1) * 512]
                        nc.vector.tensor_mul(dst, tr, rstd)
                    # V: evict transposed VT then PE-transpose to natural
                    vt = wp.tile([128, 512], TDT, name="vt", tag="vt")
                    nc.scalar.copy(vt, slabs[1][:, 1, :])
                    for j in range(4):
                        pv = psT.tile([128, 128], TDT, name="pv", tag="pv")
                        nc.tensor.transpose(pv, vt[:, j * 128:(j + 1) * 128], ident)
                        nc.scalar.copy(v_sb[:, t * 4 + j, :], pv)

            # ---------------- Phase B: causal attention + o-proj --------------
            with ExitStack() as pb:
                ep = pb.enter_context(tc.tile_pool(name="ep", bufs=20))
                wp2 = pb.enter_context(tc.tile_pool(name="wp2", bufs=3))
                atp = pb.enter_context(tc.tile_pool(name="atp", bufs=8))
                op = pb.enter_context(tc.tile_pool(name="op", bufs=4))
                psS = pb.enter_context(tc.tile_pool(name="psS", bufs=3, space="PSUM"))
                psO = pb.enter_context(tc.tile_pool(name="psO", bufs=2, space="PSUM"))
                psD = pb.enter_context(tc.tile_pool(name="psD", bufs=1, space="PSUM"))
                psP = pb.enter_context(tc.tile_pool(name="psP", bufs=2, space="PSUM"))

                for b in range(B):
                    for qt in range(QT_PER_B):
                        q0 = qt * 512
                        at_tiles = {}
                        for h in range(HQ):
                            for qh in range(2):  # 256-wide q slices
                                qq0 = q0 + qh * 256
                                n_kt = (qq0 + 256) // 128  # valid k tiles
                                # sub-phase 1: scores, two k-tiles packed
                                # per PSUM bank, one exp per pair, causal mask
                                ets = [None] * n_kt
                                for kp in range(n_kt // 2):
                                    st = psS.tile([128, 2, 256], F32, name="st", tag="st")
                                    for j in range(2):
                                        kt = 2 * kp + j
                                        nc.tensor.matmul(
                                            st[:, j, :],
                                            lhsT=(kt_sb[:, b * S + kt * 128: b * S + (kt + 1) * 128]),
                                            rhs=(qt_sb[:, h, b * S + qq0: b * S + qq0 + 256]),
                                            start=True, stop=True,
                                        )
                                    etp = ep.tile([128, 2, 256], MDT, name="et", tag="et")
                                    nc.scalar.activation(
                                        etp, st, mybir.ActivationFunctionType.Exp,
                                        scale=SCALE,
                                    )
                                    for j in range(2):
                                        kt = 2 * kp + j
                                        et = etp[:, j, :]
                                        if kt * 128 + 127 > qq0:  # diagonal band
                                            nc.gpsimd.affine_select(
                                                out=et, in_=et,
                                                pattern=[[1, 256]],
                                                channel_multiplier=-1,
                                                base=-(kt * 128 - qq0),
                                                compare_op=mybir.AluOpType.is_ge,
                                                fill=0.0,
                                            )
                                        ets[kt] = et
                                # sub-phase 2: denominator + PV accumulation
                                ot = psO.tile([128, 256], F32, name="ot", tag="ot")
                                den = psD.tile([1, 256], F32, name="den", tag="den")
                                for kt in range(n_kt):
                                    nc.tensor.matmul(
                                        den, lhsT=ones_col, rhs=ets[kt],
                                        start=(kt == 0), stop=(kt == n_kt - 1),
                                    )
                                    nc.tensor.matmul(
                                        ot, lhsT=(v_sb[:, b * (S // 128) + kt, :]),
                                        rhs=(ets[kt]),
                                        start=(kt == 0), stop=(kt == n_kt - 1),
                                    )
                                rd = wp2.tile([1, 256], F32, name="rd", tag="rd")
                                nc.vector.reciprocal(rd, den)
                                rb = wp2.tile([128, 256], F32, name="rb", tag="rb")
                                nc.gpsimd.partition_broadcast(rb, rd)
                                at = atp.tile([128, 256], MDT, name="at", tag="at")
                                nc.vector.tensor_mul(at, ot, rb)
                                at_tiles[(h, qh)] = at
                        # o-proj partial for rows [b*S+q0, +512)
                        for mq in range(4):
                            qh = mq // 2
                            mq2 = mq % 2  # 128-slice within the 256 at tile
                            for nn in range(4):
                                po = psP.tile([128, 512], F32, name="po", tag="po")
                                for h in range(HQ):
                                    nc.tensor.matmul(
                                        po,
                                        lhsT=(at_tiles[(h, qh)][:, mq2 * 128:(mq2 + 1) * 128]),
                                        rhs=(wo_sb[:, h, nn * 512:(nn + 1) * 512]),
                                        start=(h == 0), stop=(h == HQ - 1),
                                    )
                                ob = op.tile([128, 512], F32, name="ob", tag="ob")
                                # batch 1: ACT has slack (phase-A tail done) and
                                # DVE is the mid-phase-B choke; batch 0: keep DVE
                                if b == 1 and (mq + nn) % 2 == 0:
                                    nc.scalar.copy(ob, po)
                                else:
                                    nc.vector.tensor_copy(ob, po)
                                nc.sync.dma_start(
                                    out=out[b * S + q0 + mq * 128: b * S + q0 + (mq + 1) * 128,
                                            nn * 512:(nn + 1) * 512],
                                    in_=ob,
                                )
    nc.compile()
    return nc


def _rot_half(w):
    return np.concatenate([w[D // 2:], w[:D // 2]])


def prep_inputs(x, cos, sin, wq, wk, wv, wo, q_norm_w, k_norm_w):
    """Host-side sharding/layout prep. Returns per-core in_maps."""
    f = np.float32
    if KDT == "bf16":
        import ml_dtypes
        mf = np.dtype(ml_dtypes.bfloat16)
    else:
        mf = np.float32
    cvt = lambda a: np.ascontiguousarray(a.astype(mf))
    x = np.asarray(x, f)
    cos = np.asarray(cos, f)
    sin = np.asarray(sin, f)
    wq, wk, wv, wo = (np.asarray(a, f) for a in (wq, wk, wv, wo))
    q_norm_w = np.asarray(q_norm_w, f)
    k_norm_w = np.asarray(k_norm_w, f)

    xt = np.ascontiguousarray(x.reshape(T, HID).T)  # [HID, T]
    ctq = np.ascontiguousarray(cos.T * q_norm_w[:, None])
    stq = np.ascontiguousarray(sin.T * _rot_half(q_norm_w)[:, None])
    ctk = np.ascontiguousarray(cos.T * k_norm_w[:, None])
    stk = np.ascontiguousarray(sin.T * _rot_half(k_norm_w)[:, None])
    # rotate-half permutation (with sign) as a matmul stationary operand:
    # out[d] = sum_j pmat[j, d] * q[j] = sign(d) * q[(d+64) % 128]
    pmat = np.zeros((D, D), f)
    for d in range(D // 2):
        pmat[d + D // 2, d] = -1.0
    for d in range(D // 2, D):
        pmat[d - D // 2, d] = 1.0
    onec = np.ones((D, 1), f)
    xt_m, ctq_m, stq_m, ctk_m, stk_m, pmat_m, onec_m = (
        cvt(a) for a in (xt, ctq, stq, ctk, stk, pmat, onec))

    in_maps = []
    for c in range(NCORES):
        wqkv_c = np.ascontiguousarray(np.concatenate([
            wq[:, c * HQ * D:(c + 1) * HQ * D],
            wk[:, c * D:(c + 1) * D],
            wv[:, c * D:(c + 1) * D],
        ], axis=1))
        woc = np.ascontiguousarray(wo[c * HQ * D:(c + 1) * HQ * D, :])
        in_maps.append({
            "xt": xt_m, "wqkv": cvt(wqkv_c), "woc": cvt(woc),
            "pmat": pmat_m, "onec": onec_m,
            "ctq": ctq_m, "stq": stq_m, "ctk": ctk_m, "stk": stk_m,
        })
    return in_maps


_NC = None


def get_nc():
    global _NC
    if _NC is None:
        _NC = build_nc()
    return _NC


def kernel(x, cos, sin, wq, wk, wv, wo, q_norm_w, k_norm_w):
    nc = get_nc()
    in_maps = prep_inputs(x, cos, sin, wq, wk, wv, wo, q_norm_w, k_norm_w)
    res = run_bass_kernel_spmd(nc, in_maps, core_ids=list(range(NCORES)))
    acc = np.zeros((T, HID), dtype=np.float64)
    for c in range(NCORES):
        acc += res.results[c]["out"]
    return acc.astype(np.float32).reshape(B, S, HID)



# revision 4
# speedup vs baseline: 1.1388x; 1.1388x over previous
"""Trainium2 Bass kernel for a GQA attention block (B=2, S=2048, H=2048,
16 q-heads / 8 kv-heads, head_dim=128, fp32), tensor-parallel over heads
across 8 NeuronCores.

Per-core shard (core c): q-heads {2c, 2c+1}, kv-head c; wq/wk/wv column
shards, wo row shard. x is replicated (pre-transposed on host so the
contraction dim lands on SBUF partitions). Each core emits a bf16 partial
[4096, 2048] o-proj product; the host gather for the row-parallel o-proj
is a sum over the 8 partials.

All device data is bf16 (PSUM accumulation stays f32): halves HBM traffic
and enables the DVE 16-bit fast modes. DMAs are batched into whole-tile
transfers (the SP sequencer charges ~0.6-0.9us per DMA issue, so DMA
count matters as much as bytes).

Device dataflow (per core):
  A) Q^T/K^T projections in [d, tok] layout (f32 PSUM, ap-512 matmuls);
     V projected directly in natural [tok, d] layout (ap-128 matmuls,
     no PE transposes). RMSNorm sum-of-squares via GPSIMD partition
     all-reduce (q/k norm weights folded into the RoPE tables on host);
     RoPE rotate-half as a pmat matmul; rstd applied after RoPE.
  B) Causal attention per (batch, q-tile, head, 256-q-chunk):
     scores S^T [kpos,q] + exp on ACT (no max subtraction: RMSNorm
     bounds |scores| <= sqrt(128)) + causal affine_select on the
     diagonal pair; softmax denominator via bf16 pair/quad-tree adds on
     DVE + one ones-matmul per quad (PSUM-accumulated); PV with natural
     V stationary. The o-proj for q-tile T is interleaved with the
     attention chunks of q-tile T+1 to fill PE dependency bubbles.
"""

import math
import os
import sys

import numpy as np

for _p in ("/opt/trn_rl_repo", "/root/.axon_site/_ro/trn_rl_repo"):
    if os.path.isdir(_p) and _p not in sys.path:
        sys.path.insert(0, _p)
        break

import concourse.bacc as bacc
import concourse.tile as tile
from concourse import mybir
from concourse.bass_isa import ReduceOp
from concourse.bass_utils import run_bass_kernel_spmd

# Problem constants (hardcoded per contract)
B, S, HID = 2, 2048, 2048
NH, NKV, D = 16, 8, 128
NCORES = 8
HQ = NH // NCORES  # q heads per core = 2
T = B * S          # 4096 tokens
EPS = 1e-5
F32 = mybir.dt.float32
BF16 = mybir.dt.bfloat16
MDT = BF16
SCALE = 1.0 / math.sqrt(D)

KT = HID // 128      # 16 contraction tiles
TT = T // 512        # 8 token tiles of 512
QT_PER_B = S // 512  # 4 q-tiles per batch


def build_nc():
    nc = bacc.Bacc("TRN2", target_bir_lowering=False, debug=False)
    xt = nc.dram_tensor("xt", [HID, T], MDT, kind="ExternalInput").ap()
    wqkv = nc.dram_tensor("wqkv", [HID, 4 * D], MDT, kind="ExternalInput").ap()
    woc = nc.dram_tensor("woc", [HQ * D, HID], MDT, kind="ExternalInput").ap()
    pmat = nc.dram_tensor("pmat", [D, D], MDT, kind="ExternalInput").ap()
    onec = nc.dram_tensor("onec", [D, 1], MDT, kind="ExternalInput").ap()
    ctq = nc.dram_tensor("ctq", [D, S], MDT, kind="ExternalInput").ap()
    stq = nc.dram_tensor("stq", [D, S], MDT, kind="ExternalInput").ap()
    ctk = nc.dram_tensor("ctk", [D, S], MDT, kind="ExternalInput").ap()
    stk = nc.dram_tensor("stk", [D, S], MDT, kind="ExternalInput").ap()
    out = nc.dram_tensor("out", [T, HID], MDT, kind="ExternalOutput").ap()

    xt_r = xt.rearrange("(kt p) t -> p kt t", p=128)
    wqkv_r = wqkv.rearrange("(kt p) m -> p kt m", p=128)

    with tile.TileContext(nc) as tc:
        from contextlib import ExitStack

        with ExitStack() as root:
            root.enter_context(nc.allow_low_precision(
                reason="bf16 device data validated against 2e-2 rel-err gate"))
            const = root.enter_context(tc.tile_pool(name="const", bufs=1))
            ones_col = const.tile([128, 1], MDT, name="ones_col")
            nc.scalar.dma_start(out=ones_col, in_=onec)
            pmat_sb = const.tile([D, D], MDT, name="pmat_sb")
            nc.scalar.dma_start(out=pmat_sb, in_=pmat)
            eps_col = const.tile([128, 1], F32, name="eps_col")
            nc.vector.memset(eps_col, EPS)

            res = root.enter_context(tc.tile_pool(name="res", bufs=1))
            wo_sb = res.tile([128, HQ, HID], MDT, name="wo_sb")
            qt_sb = res.tile([128, HQ, T], MDT, name="qt_sb")   # [d, h, tok]
            kt_sb = res.tile([128, T], MDT, name="kt_sb")       # [d, tok]
            v_sb = res.tile([128, T // 128, D], MDT, name="v_sb")  # [tok%128, chunk, d]
            tabs = {}
            for nm in ("cq", "sq", "ck", "sk"):
                tabs[nm] = res.tile([128, S], MDT, name="tab_" + nm)

            # ---------------- Phase A: QKV^T, norm, rope ----------------------
            with ExitStack() as pa:
                wqp = pa.enter_context(tc.tile_pool(name="wqp", bufs=1))
                xp = pa.enter_context(tc.tile_pool(name="xp", bufs=3))
                wp = pa.enter_context(tc.tile_pool(name="wp", bufs=2))
                psQ = pa.enter_context(tc.tile_pool(name="psQ", bufs=2, space="PSUM"))
                psK = pa.enter_context(tc.tile_pool(name="psK", bufs=2, space="PSUM"))
                psV = pa.enter_context(tc.tile_pool(name="psV", bufs=1, space="PSUM"))
                psR = pa.enter_context(tc.tile_pool(name="psR", bufs=1, space="PSUM"))

                wqkv_sb = wqp.tile([128, KT, 4 * D], MDT, name="wqkv_sb")

                tab_srcs = {"cq": ctq, "sq": stq, "ck": ctk, "sk": stk}
                for ti, t in enumerate((0, 4, 1, 5, 2, 6, 3, 7)):
                    xk = xp.tile([128, KT, 512], MDT, name="xk", tag="xk")
                    if ti == 0:
                        # interleave wqkv / x sub-DMAs so the first matmul can
                        # start after ~3us instead of waiting for full tiles
                        for g in range(4):
                            nc.sync.dma_start(
                                out=wqkv_sb[:, 4 * g:4 * (g + 1), :],
                                in_=wqkv_r[:, 4 * g:4 * (g + 1), :],
                            )
                            nc.sync.dma_start(
                                out=xk[:, 4 * g:4 * (g + 1), :],
                                in_=xt_r[:, 4 * g:4 * (g + 1), t * 512:(t + 1) * 512],
                            )
                            if g >= 1:  # rope tables, needed by ~10us
                                for nm in (("cq",), ("sq", "ck"), ("sk",))[g - 1]:
                                    nc.sync.dma_start(out=tabs[nm], in_=tab_srcs[nm])
                    else:
                        nc.sync.dma_start(
                            out=xk, in_=xt_r[:, :, t * 512:(t + 1) * 512]
                        )
                        if ti == 2:  # wo not needed until phase B
                            nc.sync.dma_start(
                                out=wo_sb, in_=woc.rearrange("(h p) n -> p h n", p=128)
                            )

                    # Q^T projections: [d, tok] layout, ap-512
                    q01 = psQ.tile([128, 2, 512], F32, name="q01", tag="q01")
                    for k in range(KT):
                        for m in range(2):
                            nc.tensor.matmul(
                                q01[:, m, :],
                                lhsT=(wqkv_sb[:, k, m * 128:(m + 1) * 128]),
                                rhs=(xk[:, k, :]),
                                start=(k == 0), stop=(k == KT - 1),
                            )
                    kps = psK.tile([128, 512], F32, name="kps", tag="kps")
                    for k in range(KT):
                        nc.tensor.matmul(
                            kps,
                            lhsT=(wqkv_sb[:, k, 2 * 128:3 * 128]),
                            rhs=(xk[:, k, :]),
                            start=(k == 0), stop=(k == KT - 1),
                        )
                    # V directly in natural [tok, d] layout, ap-128
                    vps = psV.tile([128, 4, 128], F32, name="vps", tag="vps")
                    for c in range(4):
                        for k in range(KT):
                            nc.tensor.matmul(
                                vps[:, c, :],
                                lhsT=(xk[:, k, c * 128:(c + 1) * 128]),
                                rhs=(wqkv_sb[:, k, 3 * 128:4 * 128]),
                                start=(k == 0), stop=(k == KT - 1),
                            )
                    nc.scalar.copy(v_sb[:, t * 4:(t + 1) * 4, :], vps)

                    s0 = (t % QT_PER_B) * 512  # position-in-sequence of this tile
                    for m, cosT, sinT in (
                        (0, tabs["cq"], tabs["sq"]),
                        (1, tabs["cq"], tabs["sq"]),
                        (2, tabs["ck"], tabs["sk"]),
                    ):
                        src = q01[:, m, :] if m < 2 else kps
                        qk = wp.tile([128, 512], MDT, name="qk", tag="qk")
                        nc.scalar.copy(qk, src)  # sole PSUM reader (ACT)
                        sq = wp.tile([128, 512], MDT, name="sq", tag="sq")
                        nc.vector.tensor_mul(sq, qk, qk)
                        nc.gpsimd.partition_all_reduce(sq, sq, 128, ReduceOp.add)
                        rrow = wp.tile([1, 512], F32, name="rrow", tag="rrow")
                        nc.scalar.activation(
                            rrow, sq[0:1, :], mybir.ActivationFunctionType.Sqrt,
                            bias=eps_col[0:1, :], scale=1.0 / D,
                        )
                        rinv = wp.tile([1, 512], MDT, name="rinv", tag="rinv")
                        nc.vector.reciprocal(rinv, rrow)
                        rstd = wp.tile([128, 512], MDT, name="rstd", tag="rstd")
                        nc.gpsimd.partition_broadcast(rstd, rinv)
                        shf = psR.tile([128, 512], F32, name="shf", tag="shf")
                        nc.tensor.matmul(shf, lhsT=pmat_sb, rhs=qk, start=True, stop=True)
                        t0 = wp.tile([128, 512], MDT, name="t0", tag="t0")
                        nc.vector.tensor_mul(t0, qk, cosT[:, s0:s0 + 512])
                        t1 = wp.tile([128, 512], MDT, name="t1", tag="t1")
                        nc.vector.tensor_mul(t1, shf, sinT[:, s0:s0 + 512])
                        tr = wp.tile([128, 512], MDT, name="tr", tag="tr")
                        nc.vector.tensor_add(tr, t0, t1)
                        if m < 2:
                            dst = qt_sb[:, m, t * 512:(t + 1) * 512]
                        else:
                            dst = kt_sb[:, t * 512:(t + 1) * 512]
                        nc.vector.tensor_mul(dst, tr, rstd)

            # ---------------- Phase B: causal attention + o-proj --------------
            with ExitStack() as pb:
                ep = pb.enter_context(tc.tile_pool(name="ep", bufs=10))
                wp2 = pb.enter_context(tc.tile_pool(name="wp2", bufs=4))
                sump = pb.enter_context(tc.tile_pool(name="sump", bufs=6))
                atp = pb.enter_context(tc.tile_pool(name="atp", bufs=8))
                op = pb.enter_context(tc.tile_pool(name="op", bufs=2))
                psS = pb.enter_context(tc.tile_pool(name="psS", bufs=4, space="PSUM"))
                psOD = pb.enter_context(tc.tile_pool(name="psOD", bufs=2, space="PSUM"))
                psP = pb.enter_context(tc.tile_pool(name="psP", bufs=2, space="PSUM"))

                def emit_scores(b, qt, h, qh):
                    """Sub-phase 1: score matmuls + exp + causal mask.
                    Returns (etps, n_kt): list of [128,2,256] bf16 exp tiles."""
                    q0 = qt * 512
                    qq0 = q0 + qh * 256
                    n_kt = (qq0 + 256) // 128
                    etps = []
                    for kp in range(n_kt // 2):
                        st = psS.tile([128, 2, 256], F32, name="st", tag="st")
                        for j in range(2):
                            kt = 2 * kp + j
                            nc.tensor.matmul(
                                st[:, j, :],
                                lhsT=(kt_sb[:, b * S + kt * 128: b * S + (kt + 1) * 128]),
                                rhs=(qt_sb[:, h, b * S + qq0: b * S + qq0 + 256]),
                                start=True, stop=True,
                            )
                        etp = ep.tile([128, 2, 256], MDT, name="et", tag="et")
                        nc.scalar.activation(
                            etp, st, mybir.ActivationFunctionType.Exp, scale=SCALE,
                        )
                        if kp == n_kt // 2 - 1:  # diagonal pair: causal mask
                            for j in range(2):
                                kt = 2 * kp + j
                                nc.gpsimd.affine_select(
                                    out=etp[:, j, :], in_=etp[:, j, :],
                                    pattern=[[1, 256]],
                                    channel_multiplier=-1,
                                    base=-(kt * 128 - qq0),
                                    compare_op=mybir.AluOpType.is_ge,
                                    fill=0.0,
                                )
                        etps.append(etp)
                    return etps, n_kt

                def emit_denpv(b, qt, h, qh, etps, n_kt):
                    """Sub-phase 2: den tree + PV accumulation + at scaling."""
                    n_pair = n_kt // 2
                    # bf16 pair sums on DVE, then quad sums; ones-matmul per
                    # quad (or leftover pair), PSUM-accumulated => exact f32 den
                    psums = []
                    for kp in range(n_pair):
                        psum = sump.tile([128, 256], MDT, name="psum", tag="psum")
                        nc.vector.tensor_add(psum, etps[kp][:, 0, :], etps[kp][:, 1, :])
                        psums.append(psum)
                    dsums = []
                    for qd in range(n_pair // 2):
                        qsum = sump.tile([128, 256], MDT, name="qsum", tag="qsum")
                        nc.vector.tensor_add(qsum, psums[2 * qd], psums[2 * qd + 1])
                        dsums.append(qsum)
                    if n_pair % 2:
                        dsums.append(psums[-1])
                    od = psOD.tile([128, 2, 256], F32, name="od", tag="od")
                    ot = od[:, 0, :]
                    den = od[0:1, 1, :]
                    for i, dsum in enumerate(dsums):
                        nc.tensor.matmul(
                            den, lhsT=ones_col, rhs=dsum,
                            start=(i == 0), stop=(i == len(dsums) - 1),
                        )
                    for kt in range(n_kt):
                        nc.tensor.matmul(
                            ot, lhsT=(v_sb[:, b * (S // 128) + kt, :]),
                            rhs=(etps[kt // 2][:, kt % 2, :]),
                            start=(kt == 0), stop=(kt == n_kt - 1),
                        )
                    rd = wp2.tile([1, 256], MDT, name="rd", tag="rd")
                    nc.vector.reciprocal(rd, den)
                    rb = wp2.tile([128, 256], MDT, name="rb", tag="rb")
                    nc.gpsimd.partition_broadcast(rb, rd)
                    at = atp.tile([128, 256], MDT, name="at", tag="at")
                    nc.vector.tensor_mul(at, ot, rb)
                    return at

                def emit_oproj_part(b, qt, at_tiles, mq):
                    """o-proj matmuls + eviction + store for one 128-row block."""
                    q0 = qt * 512
                    qh = mq // 2
                    mq2 = mq % 2
                    ob = op.tile([128, 4, 512], MDT, name="ob", tag="ob")
                    for nn in range(4):
                        po = psP.tile([128, 512], F32, name="po", tag="po")
                        for h in range(HQ):
                            nc.tensor.matmul(
                                po,
                                lhsT=(at_tiles[(h, qh)][:, mq2 * 128:(mq2 + 1) * 128]),
                                rhs=(wo_sb[:, h, nn * 512:(nn + 1) * 512]),
                                start=(h == 0), stop=(h == HQ - 1),
                            )
                        if nn % 2 == 0:
                            nc.scalar.copy(ob[:, nn, :], po)
                        else:
                            nc.vector.tensor_copy(ob[:, nn, :], po)
                    nc.sync.dma_start(
                        out=out[b * S + q0 + mq * 128: b * S + q0 + (mq + 1) * 128, :],
                        in_=ob,
                    )

                # chunk-level software pipeline: scores(c+1) issued before
                # den/pv(c); o-proj of the previous q-tile interleaved between
                # chunks to fill PE dependency bubbles
                prev = None  # (b, qt, at_tiles)
                for b in range(B):
                    for qt in range(QT_PER_B):
                        chunks = [(h, qh) for h in range(HQ) for qh in range(2)]
                        at_tiles = {}
                        pend = []  # emitted scores awaiting den/pv
                        for ci, (h, qh) in enumerate(chunks):
                            etps, n_kt = emit_scores(b, qt, h, qh)
                            pend.append((h, qh, etps, n_kt))
                            if ci >= 1:
                                ph, pqh, petps, pn = pend.pop(0)
                                at_tiles[(ph, pqh)] = emit_denpv(b, qt, ph, pqh, petps, pn)
                            if prev is not None:
                                pb_, pqt, pat = prev
                                if ci == 2:
                                    emit_oproj_part(pb_, pqt, pat, 0)
                                    emit_oproj_part(pb_, pqt, pat, 1)
                                elif ci == 3:
                                    emit_oproj_part(pb_, pqt, pat, 2)
                                    emit_oproj_part(pb_, pqt, pat, 3)
                        ph, pqh, petps, pn = pend.pop(0)
                        at_tiles[(ph, pqh)] = emit_denpv(b, qt, ph, pqh, petps, pn)
                        prev = (b, qt, at_tiles)
                # drain the last q-tile's o-proj
                pb_, pqt, pat = prev
                for mq in range(4):
                    emit_oproj_part(pb_, pqt, pat, mq)
    nc.compile()
    return nc


def _rot_half(w):
    return np.concatenate([w[D // 2:], w[:D // 2]])


def prep_inputs(x, cos, sin, wq, wk, wv, wo, q_norm_w, k_norm_w):
    """Host-side sharding/layout prep. Returns per-core in_maps."""
    import ml_dtypes
    f = np.float32
    mf = np.dtype(ml_dtypes.bfloat16)
    cvt = lambda a: np.ascontiguousarray(a.astype(mf))
    x = np.asarray(x, f)
    cos = np.asarray(cos, f)
    sin = np.asarray(sin, f)
    wq, wk, wv, wo = (np.asarray(a, f) for a in (wq, wk, wv, wo))
    q_norm_w = np.asarray(q_norm_w, f)
    k_norm_w = np.asarray(k_norm_w, f)

    xt = np.ascontiguousarray(x.reshape(T, HID).T)  # [HID, T]
    ctq = np.ascontiguousarray(cos.T * q_norm_w[:, None])
    stq = np.ascontiguousarray(sin.T * _rot_half(q_norm_w)[:, None])
    ctk = np.ascontiguousarray(cos.T * k_norm_w[:, None])
    stk = np.ascontiguousarray(sin.T * _rot_half(k_norm_w)[:, None])
    # rotate-half permutation (with sign) as a matmul stationary operand:
    # out[d] = sum_j pmat[j, d] * q[j] = sign(d) * q[(d+64) % 128]
    pmat = np.zeros((D, D), f)
    for d in range(D // 2):
        pmat[d + D // 2, d] = -1.0
    for d in range(D // 2, D):
        pmat[d - D // 2, d] = 1.0
    onec = np.ones((D, 1), f)
    xt_m, ctq_m, stq_m, ctk_m, stk_m, pmat_m, onec_m = (
        cvt(a) for a in (xt, ctq, stq, ctk, stk, pmat, onec))

    in_maps = []
    for c in range(NCORES):
        wqkv_c = np.ascontiguousarray(np.concatenate([
            wq[:, c * HQ * D:(c + 1) * HQ * D],
            wk[:, c * D:(c + 1) * D],
            wv[:, c * D:(c + 1) * D],
        ], axis=1))
        woc = np.ascontiguousarray(wo[c * HQ * D:(c + 1) * HQ * D, :])
        in_maps.append({
            "xt": xt_m, "wqkv": cvt(wqkv_c), "woc": cvt(woc),
            "pmat": pmat_m, "onec": onec_m,
            "ctq": ctq_m, "stq": stq_m, "ctk": ctk_m, "stk": stk_m,
        })
    return in_maps


_NC = None


def get_nc():
    global _NC
    if _NC is None:
        _NC = build_nc()
    return _NC


def kernel(x, cos, sin, wq, wk, wv, wo, q_norm_w, k_norm_w):
    nc = get_nc()
    in_maps = prep_inputs(x, cos, sin, wq, wk, wv, wo, q_norm_w, k_norm_w)
    res = run_bass_kernel_spmd(nc, in_maps, core_ids=list(range(NCORES)))
    acc = np.zeros((T, HID), dtype=np.float64)
    for c in range(NCORES):
        acc += np.asarray(res.results[c]["out"], dtype=np.float64)
    return acc.astype(np.float32).reshape(B, S, HID)
